# revision 1
# baseline (speedup 1.0000x reference)
"""Causal multi-head attention (B=2, T=2048, D=1024, H=16) on 8 TRN2 NeuronCores.

Sharding: core c = (batch b = c//4, head-group g = c%4). Each core owns 4 heads
(= 256 contiguous dims of D) of one batch: Megatron-style tensor parallelism on
heads x data parallelism on batch. Per-core partial output projections are
summed with chunked on-chip ReduceScatters over each batch's 4 cores; the host
only re-assembles the resulting shards.

Device-side layout choices (host pre-transposes, pure data movement):
  - xT  [D, T]        = x[b].T so projections contract D on the partition dim.
  - qT/kT [256, T]    computed directly transposed (dims on partitions).
  - scoresT[k, q]     = k @ qT -> softmax runs in the k-on-partitions domain,
                        so the AV matmul (lhsT=v, rhs=attnT) needs no T x T
                        transpose anywhere.
  - v_aug [k, 4*65]   v with a ones column appended per head: AV then yields
                        yT' [65, span] whose row 64 is the softmax denominator.
  - softmax: exp(s) without row-max subtraction (scores are O(1): the q,k
    projections are variance-1, scale 1/8 folded into Wq host-side), causal
    tile classification (full-skip / full-keep / diagonal-with-mask-values).
  - normalization: per-span stacked reciprocal on DVE, broadcast across
    partitions via a PE rank-1 outer product, applied during the PSUM->SBUF
    evacuation of yT'.
  - per-q-span pipeline: attention -> normalize -> out-projection -> chunked
    ReduceScatter -> output DMA, so collectives overlap the next span.

Dtypes: all matmul operands run in bf16 (1.0 PE cycles/row; f32r would be 1.5)
with fp32 PSUM accumulation throughout; biases are added in fp32 during PSUM
evacuation. The softmax normalization cancels most of the correlated bf16
quantization error: measured end-to-end relative error is ~5.9e-3 vs the fp32
reference (absmax ~0.4% of the output scale), verified identically in CoreSim
and on hardware.
"""

import os
import numpy as np
import ml_dtypes

BF16 = ml_dtypes.bfloat16

B, T, D, H = 2, 2048, 1024, 16
HD = D // H                     # 64
NCORES = 8
GROUPS = 4                      # cores per batch (tensor-parallel degree)
HL = H // GROUPS                # heads per core = 4
DL = D // GROUPS                # dims per core = 256
SP = 512                        # free-dim span per matmul (one PSUM bank, fp32)
QS = T // SP                    # 4 q spans
KT = T // 128                   # 16 k tiles
RS_ROWS = T // GROUPS           # 512 rows per ReduceScatter chunk
SCALE = HD ** -0.5

_CACHE = {}


def _build_program():
    import concourse.bass as bass  # noqa: F401  (registers bass machinery)
    import concourse.tile as tile
    from concourse import bacc, mybir

    f32 = mybir.dt.float32
    f32r = mybir.dt.float32r
    bf16 = mybir.dt.bfloat16
    Exp = mybir.ActivationFunctionType.Exp
    Identity = mybir.ActivationFunctionType.Identity

    nc = bacc.Bacc("TRN2", target_bir_lowering=False, debug=False,
                   num_devices=NCORES)

    xT = nc.dram_tensor("xT", [D, T], bf16, kind="ExternalInput")
    wqT = nc.dram_tensor("wqT", [D, DL], bf16, kind="ExternalInput")
    wkT = nc.dram_tensor("wkT", [D, DL], bf16, kind="ExternalInput")
    wvT = nc.dram_tensor("wvT", [D, DL], bf16, kind="ExternalInput")
    woT = nc.dram_tensor("woT", [DL, D], bf16, kind="ExternalInput")
    bqP = nc.dram_tensor("bqP", [128, 2], f32, kind="ExternalInput")
    bkP = nc.dram_tensor("bkP", [128, 2], f32, kind="ExternalInput")
    bv = nc.dram_tensor("bv", [1, DL], bf16, kind="ExternalInput")
    bo = nc.dram_tensor("bo", [1, D], bf16, kind="ExternalInput")
    maskd = nc.dram_tensor("maskd", [KT, 128, SP], bf16, kind="ExternalInput")
    onesd = nc.dram_tensor("onesd", [128, SP], f32r, kind="ExternalInput")
    onesb = nc.dram_tensor("onesb", [128, SP], bf16, kind="ExternalInput")
    out_ext = nc.dram_tensor("out", [QS, 128, D], f32, kind="ExternalOutput")

    with tile.TileContext(nc) as tc:
        with tc.tile_pool(name="main", bufs=1) as main, \
             tc.tile_pool(name="dram", bufs=1, space="DRAM") as dram:
            qT_s = main.tile([128, 2, T], bf16)
            kT_s = main.tile([128, 2, T], bf16)
            v_s = main.tile([128, KT, HL * 65], bf16)
            yT_s = main.tile([128, 2, T], bf16)
            woT_s = main.tile([128, 2, D], bf16)
            bq_s = main.tile([128, 2], f32)
            bk_s = main.tile([128, 2], f32)
            bv_s = main.tile([1, DL], bf16)
            bo_s = main.tile([1, D], bf16)
            ones_s = main.tile([128, SP], f32r)
            onesb_s = main.tile([128, SP], bf16)
            bo_bc = main.tile([128, D], bf16)
            bv_bc = main.tile([128, DL], bf16)
            maskd_s = main.tile([128, KT, SP], bf16)

            # one partial/rs tile pair per q-span: avoids false DRAM-tile
            # dependencies between a span's ReduceScatter and the next
            # span's out-projection DMAs
            partials = [dram.tile([RS_ROWS, D], f32, name=f"partial{i}")
                        for i in range(QS)]
            rs_outs = [dram.tile([128, D], f32, name=f"rsout{i}")
                       for i in range(QS)]

            # tiny high-priority loads on the sync queue
            nc.sync.dma_start(out=bq_s, in_=bqP[:])
            nc.sync.dma_start(out=bk_s, in_=bkP[:])
            # small loads on the scalar queue
            nc.scalar.dma_start(out=ones_s, in_=onesd[:])
            nc.scalar.dma_start(out=onesb_s, in_=onesb[:])
            nc.scalar.dma_start(out=bv_bc, in_=bv[:].to_broadcast([128, DL]))
            nc.scalar.dma_start(out=bo_bc, in_=bo[:].to_broadcast([128, D]))
            # ones column at index 64 of each head's 65-wide block of v_aug:
            # memset the whole tile (bf16 memset is codegen-legal; the v
            # evacuations overwrite the data columns)
            nc.vector.memset(v_s, 1.0)

            # ---------------- phase 1: projections ----------------
            with tc.tile_pool(name="proj", bufs=1) as proj, \
                 tc.tile_pool(name="pj_psum", bufs=3, space="PSUM") as pj_psum:
                xt_s = proj.tile([128, 8, T], bf16)
                wq_s = proj.tile([128, 8, DL], bf16)
                wk_s = proj.tile([128, 8, DL], bf16)
                wv_s = proj.tile([128, 8, DL], bf16)

                # critical path first: wq then the x chunks (split across the
                # sync and gpsimd queues); wk/wv follow behind x on gpsimd
                wq_r = wqT[:].rearrange("(c p) n -> c p n", p=128)
                for c in range(8):
                    nc.sync.dma_start(out=wq_s[:, c, :], in_=wq_r[c])
                xT_r = xT[:].rearrange("(c p) t -> c p t", p=128)
                for c in range(8):
                    eng = nc.sync if c % 2 == 0 else nc.gpsimd
                    eng.dma_start(out=xt_s[:, c, :], in_=xT_r[c])
                # wk/wv on the scalar queue (needed only after qT finishes),
                # followed by the attention/outproj bulk loads
                for w_s, w_d in ((wk_s, wkT), (wv_s, wvT)):
                    w_r = w_d[:].rearrange("(c p) n -> c p n", p=128)
                    for c in range(8):
                        nc.scalar.dma_start(out=w_s[:, c, :], in_=w_r[c])
                for i in range(KT):
                    nc.scalar.dma_start(out=maskd_s[:, i, :], in_=maskd[i])
                woT_r = woT[:].rearrange("(c p) n -> c p n", p=128)
                for c in range(2):
                    nc.scalar.dma_start(out=woT_s[:, c, :], in_=woT_r[c])

                # qT / kT: out[dims-chunk, t-span]; bias added during the
                # PSUM->SBUF evacuation (per-partition scalar)
                for w_s, b_s, dst, use_act in ((wq_s, bq_s, qT_s, True),
                                               (wk_s, bk_s, kT_s, False)):
                    for mc in range(2):
                        for s in range(QS):
                            ps = pj_psum.tile([128, SP], f32, tag="pj")
                            for kc in range(8):
                                nc.tensor.matmul(
                                    ps,
                                    lhsT=w_s[:, kc, mc * 128:(mc + 1) * 128],
                                    rhs=xt_s[:, kc, s * SP:(s + 1) * SP],
                                    start=(kc == 0), stop=(kc == 7))
                            dstv = dst[:, mc, s * SP:(s + 1) * SP]
                            if use_act:
                                nc.scalar.activation(
                                    dstv, ps, Identity,
                                    bias=b_s[:, mc:mc + 1])
                            else:
                                nc.vector.tensor_scalar_add(
                                    dstv, ps, b_s[:, mc:mc + 1])

                # v: natural layout; bias via rank-1 matmul (free-dim bias)
                for mt in range(KT):
                    ps = pj_psum.tile([128, DL], f32, tag="pjv")
                    for kc in range(8):
                        nc.tensor.matmul(
                            ps,
                            lhsT=xt_s[:, kc, mt * 128:(mt + 1) * 128],
                            rhs=wv_s[:, kc, :],
                            start=(kc == 0), stop=(kc == 7))
                    nc.vector.tensor_add(
                        v_s[:, mt, :].rearrange(
                            "p (h d) -> p h d", d=65)[:, :, 0:64],
                        ps.rearrange("p (h d) -> p h d", d=64),
                        bv_bc.rearrange("p (h d) -> p h d", d=64))

            # ---- phase 2: per-span attention, software-pipelined with the
            # previous span's normalize-broadcast + out-projection + RS so
            # the in-order PE queue never waits on the DVE normalize chain
            with tc.tile_pool(name="attn_t", bufs=6) as attn_t, \
                 tc.tile_pool(name="nrm", bufs=2) as nrm, \
                 tc.tile_pool(name="op_sb", bufs=4) as op_sb, \
                 tc.tile_pool(name="sc_psum", bufs=2, space="PSUM") as sc_psum, \
                 tc.tile_pool(name="av_psum", bufs=3, space="PSUM") as av_psum, \
                 tc.tile_pool(name="pp_psum", bufs=3, space="PSUM") as pp_psum:

                def attention_span(qs):
                    # denominator rows at partitions 0/32/64/96 (engine APs
                    # must start 32-aligned); memset keeps unused rows finite
                    den_stack = nrm.tile([97, SP], f32, tag="den")
                    nc.vector.memset(den_stack, 1.0)
                    nkt = 4 * qs + 4  # causal: later k tiles are all-masked
                    for h in range(HL):
                        mc, r0 = divmod(h, 2)
                        r0 *= 64
                        qv = qT_s[r0:r0 + 64, mc, qs * SP:(qs + 1) * SP]
                        yT_ps = av_psum.tile([65, SP], f32, tag="av")
                        for kt in range(nkt):
                            sc = sc_psum.tile([128, SP], f32, tag="sc")
                            nc.tensor.matmul(
                                sc,
                                lhsT=kT_s[r0:r0 + 64, mc,
                                          kt * 128:(kt + 1) * 128],
                                rhs=qv, start=True, stop=True)
                            at = attn_t.tile([128, SP], bf16, tag="at")
                            nc.scalar.activation(at, sc, Exp)
                            if kt >= 4 * qs:  # diagonal tile: apply mask
                                nc.vector.tensor_mul(at, at, maskd_s[:, kt, :])
                            nc.tensor.matmul(
                                yT_ps, lhsT=v_s[:, kt, h * 65:(h + 1) * 65],
                                rhs=at, start=(kt == 0), stop=(kt == nkt - 1))
                        # evacuate yT' (unnormalized) right away so the PSUM
                        # accumulator frees for the next head
                        nc.scalar.copy(
                            yT_s[r0:r0 + 64, mc, qs * SP:(qs + 1) * SP],
                            yT_ps[0:64, :])
                        nc.vector.tensor_copy(den_stack[32 * h:32 * h + 1, :],
                                              yT_ps[64:65, :])
                    # pure-DVE tail: reciprocal + per-head f32r rows for the
                    # PE broadcast (consumed one span later)
                    rec_f = nrm.tile([97, SP], f32, tag="recf")
                    nc.vector.reciprocal(rec_f, den_stack)
                    rec_hs = []
                    for h in range(HL):
                        rec_h = nrm.tile([1, SP], bf16, tag="rech", bufs=8)
                        nc.vector.tensor_copy(rec_h,
                                              rec_f[32 * h:32 * h + 1, :])
                        rec_hs.append(rec_h)
                    return rec_hs

                def pe_post(qs, rec_hs):
                    # broadcast 1/denom across partitions on the PE, then
                    # normalize yT in place
                    for h in range(HL):
                        mc, r0 = divmod(h, 2)
                        r0 *= 64
                        rb = pp_psum.tile([64, SP], f32, tag="pp")
                        nc.tensor.matmul(rb, lhsT=onesb_s[0:1, 0:64],
                                         rhs=rec_hs[h], start=True, stop=True)
                        yv = yT_s[r0:r0 + 64, mc, qs * SP:(qs + 1) * SP]
                        nc.vector.tensor_mul(yv, yv, rb)
                    # out-projection for this span's 4 q-tiles; each 256-row
                    # half's ReduceScatter fires as soon as its 2 q-tiles
                    # are written so the tail chunk starts earlier
                    hr = RS_ROWS // 2
                    for hf in range(2):
                        for qt in range(4 * qs + 2 * hf, 4 * qs + 2 * hf + 2):
                            for ns in range(2):
                                po = pp_psum.tile([128, SP], f32, tag="pp")
                                for kc in range(2):
                                    nc.tensor.matmul(
                                        po,
                                        lhsT=yT_s[:, kc,
                                                  qt * 128:(qt + 1) * 128],
                                        rhs=woT_s[:, kc,
                                                  ns * SP:(ns + 1) * SP],
                                        start=(kc == 0), stop=(kc == 1))
                                ob = op_sb.tile([128, SP], f32, tag="ob")
                                nc.vector.tensor_add(
                                    ob, po, bo_bc[:, ns * SP:(ns + 1) * SP])
                                nc.sync.dma_start(
                                    out=partials[qs][
                                        (qt - 4 * qs) * 128:
                                        (qt - 4 * qs + 1) * 128,
                                        ns * SP:(ns + 1) * SP],
                                    in_=ob)
                        nc.gpsimd.collective_compute(
                            "ReduceScatter", mybir.AluOpType.add,
                            replica_groups=[[0, 1, 2, 3], [4, 5, 6, 7]],
                            ins=[partials[qs][hf * hr:(hf + 1) * hr, :].opt()],
                            outs=[rs_outs[qs][hf * 64:(hf + 1) * 64, :].opt()])
                        nc.sync.dma_start(
                            out=out_ext[qs, hf * 64:(hf + 1) * 64, :],
                            in_=rs_outs[qs][hf * 64:(hf + 1) * 64, :])

                prev = None
                for qs in range(QS):
                    rec_hs = attention_span(qs)
                    if prev is not None:
                        pe_post(prev[0], prev[1])
                    prev = (qs, rec_hs)
                pe_post(prev[0], prev[1])

    nc.compile()
    return nc


def _get_program():
    if "nc" not in _CACHE:
        _CACHE["nc"] = _build_program()
    return _CACHE["nc"]


def _make_in_maps(x, mask, Wq, bq, Wk, bk, Wv, bv, Wo, bo):
    x = np.asarray(x, np.float32)
    mask = np.asarray(mask, bool)
    Wq = np.asarray(Wq, np.float32)
    Wk = np.asarray(Wk, np.float32)
    Wv = np.asarray(Wv, np.float32)
    Wo = np.asarray(Wo, np.float32)
    bq = np.asarray(bq, np.float32)
    bk = np.asarray(bk, np.float32)
    bv = np.asarray(bv, np.float32)
    bo = np.asarray(bo, np.float32)

    zeros_bo = np.zeros((1, D), np.float32)
    in_maps = []
    per_batch = {}
    for b in range(B):
        xTb = np.ascontiguousarray(x[b].T)
        # diagonal mask tiles of mask[b,0].T: index qs*4+j holds
        # maskT[128*(4qs+j) : +128, 512*qs : +512]
        mT = mask[b, 0].T
        md = np.empty((KT, 128, SP), np.float32)
        for qs in range(QS):
            for j in range(4):
                kt = 4 * qs + j
                md[kt] = mT[kt * 128:(kt + 1) * 128,
                            qs * SP:(qs + 1) * SP].astype(np.float32)
        per_batch[b] = (xTb, md)
    for c in range(NCORES):
        b, g = divmod(c, GROUPS)
        sl = slice(g * DL, (g + 1) * DL)
        xTb, md = per_batch[b]
        in_maps.append({
            "xT": xTb.astype(BF16),
            "wqT": np.ascontiguousarray((Wq[sl] * SCALE).T).astype(BF16),
            "wkT": np.ascontiguousarray(Wk[sl].T).astype(BF16),
            "wvT": np.ascontiguousarray(Wv[sl].T).astype(BF16),
            "woT": np.ascontiguousarray(Wo[:, sl].T).astype(BF16),
            "bqP": np.ascontiguousarray((bq[sl] * SCALE).reshape(2, 128).T),
            "bkP": np.ascontiguousarray(bk[sl].reshape(2, 128).T),
            "bv": bv[sl].reshape(1, DL).astype(BF16),
            "bo": (bo.reshape(1, D) if g == 0 else zeros_bo).astype(BF16),
            "maskd": md.astype(BF16),
            "onesd": np.ones((128, SP), np.float32),
            "onesb": np.ones((128, SP), BF16),
        })
    return in_maps


def _capture_profile(nc, in_maps, tmpdir):
    """Run with NTFF capture and process the profile ourselves (the stock
    trace path can't handle the duplicate-executable NTFFs the axon relay
    produces). Returns (results, exec_time_ns|None)."""
    import glob
    import json
    import re
    import subprocess
    from trn_agent_boot.trn_boot import _ntff_profile_via_ctypes
    from concourse import bass2jax

    hook = _ntff_profile_via_ctypes("/opt/axon/libaxon_pjrt.so")
    if hook is None:
        raise RuntimeError("libaxon_pjrt.so lacks NTFF profile symbols")
    os.makedirs(tmpdir, exist_ok=True)
    with hook(tmpdir, [0]):
        results = bass2jax.run_bass_via_pjrt(nc, in_maps, n_cores=NCORES)

    # group NTFF/NEFF pairs by executable id; use the newest executable
    ntffs = glob.glob(os.path.join(tmpdir, "*_body*-device*.ntff"))
    best, best_id = None, -1
    for f in ntffs:
        m = re.search(r"executable(\d+)-device000000", f)
        if m and int(m.group(1)) > best_id:
            best_id, best = int(m.group(1)), f
    if best is None:
        raise RuntimeError(f"no NTFF produced in {tmpdir}")
    neff = re.sub(r"-device\d+-execution-\d+\.ntff$", ".neff", best)
    out_json = os.path.join(tmpdir, "prof.json")
    subprocess.check_call(
        ["neuron-profile", "view", "--ignore-nc-buf-usage", "-s", best,
         "-n", neff, "--output-format=json", f"--output-file={out_json}"],
        cwd=tmpdir)
    summary = json.load(open(out_json))["summary"][0]
    return results, int(summary["total_time"] * 1e9)


def kernel(x, mask, Wq, bq, Wk, bk, Wv, bv, Wo, bo):
    from concourse import bass_utils

    in_maps = _make_in_maps(x, mask, Wq, bq, Wk, bk, Wv, bv, Wo, bo)
    nc = _get_program()

    trace = bool(int(os.environ.get("MHA_TRACE", "0")))
    tmpdir = os.environ.get("MHA_TRACE_DIR") or None
    results = None
    if trace and tmpdir:
        try:
            results, exec_ns = _capture_profile(nc, in_maps, tmpdir)
            _CACHE["last_exec_time_ns"] = exec_ns
        except Exception as e:  # profiling is best-effort
            print(f"profiling unavailable: {type(e).__name__}: {e}")
            results = None
    if results is None:
        results = bass_utils.run_bass_kernel_spmd(
            nc, in_maps, core_ids=list(range(NCORES))).results
        _CACHE.setdefault("last_exec_time_ns", None)

    out = np.empty((B, T, D), np.float32)
    for c in range(NCORES):
        b, rk = divmod(c, GROUPS)
        o = results[c]["out"]
        for qs in range(QS):  # each span was reduce-scattered in two halves
            for hf in range(2):
                lo = qs * RS_ROWS + hf * (RS_ROWS // 2) + rk * 64
                out[b, lo:lo + 64] = o[qs, hf * 64:(hf + 1) * 64]
    return out



# revision 11
# speedup vs baseline: 1.1927x; 1.1927x over previous
"""Causal multi-head attention (B=2, T=2048, D=1024, H=16) on 8 TRN2 NeuronCores.

Sharding: core c = (batch b = c//4, head-group g = c%4); each core owns 4 heads
(256 dims) of one batch. Partial out-projections are summed with one fp16
ReduceScatter per q-span over each batch's 4 cores.

v2 design (vs the 352us baseline):
  - software-pipelined score->exp->AV loop (score kt+1 issues before AV kt) so
    the PE never waits the ~630ns exp; ACT and PE run at parity.
  - causal column trimming: diagonal k-tiles only compute/exp/AV columns
    >= 128*kt - span_base; the partially-masked 128x128 block is the SAME
    upper-triangular pattern for every tile -> one 32KB tri tile replaces the
    2MB mask load.
  - 5 uneven q-spans (512,512,512,384,128): the last ReduceScatter is 4x
    smaller, shrinking the un-overlappable collective tail.
  - fp16 partials + one RS per span (half the collective bytes of fp32, and
    fp16 keeps quantization at 2^-11 so accuracy is unchanged).
  - projections of span s+1 and post-processing (normalize/out-proj/RS) of
    span s-1 are interleaved between the attention heads of span s, filling
    PE bubbles in ACT-bound spans and ACT bubbles in PE-bound spans.
  - per-head softmax denominators via DVE reciprocal_approx_fast straight from
    PSUM row 64 (the v-augmentation ones-column), broadcast across partitions
    with a rank-1 f32r matmul.
  - q/k bias evacuations on ACT (Identity+bias shares the exp table; no act
    table reloads), yT/v/out evacuations on DVE.
"""

import os
import numpy as np
import ml_dtypes

BF16 = ml_dtypes.bfloat16
FP16 = np.float16

B, T, D, H = 2, 2048, 1024, 16
HD = D // H                     # 64
NCORES = 8
GROUPS = 4                      # cores per batch (tensor-parallel degree)
HL = H // GROUPS                # heads per core = 4
DL = D // GROUPS                # dims per core = 256
SCALE = HD ** -0.5

WS = [512, 512, 512, 384, 128]  # q-span widths (sum = T)
BS = [0, 512, 1024, 1536, 1920]  # q-span base offsets
NSP = len(WS)

_CACHE = {}


def _build_program():
    import concourse.bass as bass  # noqa: F401  (registers bass machinery)
    import concourse.tile as tile
    from concourse import bacc, mybir

    f32 = mybir.dt.float32
    f32r = mybir.dt.float32r
    bf16 = mybir.dt.bfloat16
    fp16 = mybir.dt.float16
    Exp = mybir.ActivationFunctionType.Exp
    Identity = mybir.ActivationFunctionType.Identity

    nc = bacc.Bacc("TRN2", target_bir_lowering=False, debug=False,
                   num_devices=NCORES)

    xT = nc.dram_tensor("xT", [D, T], bf16, kind="ExternalInput")
    wqT = nc.dram_tensor("wqT", [D, DL], bf16, kind="ExternalInput")
    wkT = nc.dram_tensor("wkT", [D, DL], bf16, kind="ExternalInput")
    wvT = nc.dram_tensor("wvT", [D, DL], bf16, kind="ExternalInput")
    woT = nc.dram_tensor("woT", [DL, D], bf16, kind="ExternalInput")
    bqP = nc.dram_tensor("bqP", [128, 2], f32, kind="ExternalInput")
    bkP = nc.dram_tensor("bkP", [128, 2], f32, kind="ExternalInput")
    bv = nc.dram_tensor("bv", [1, DL], bf16, kind="ExternalInput")
    bo = nc.dram_tensor("bo", [1, D], bf16, kind="ExternalInput")
    trid = nc.dram_tensor("trid", [128, 128], bf16, kind="ExternalInput")
    out_ext = nc.dram_tensor("out", [T // GROUPS, D], f32,
                             kind="ExternalOutput")
    DBG = bool(int(os.environ.get("MHA_DEBUG", "0")))
    if DBG:
        dbg_q = nc.dram_tensor("dbg_q", [128, 2, T], mybir.dt.bfloat16,
                               kind="ExternalOutput")
        dbg_k = nc.dram_tensor("dbg_k", [128, 2, T], mybir.dt.bfloat16,
                               kind="ExternalOutput")
        dbg_v = nc.dram_tensor("dbg_v", [128, 16, HL * 65], mybir.dt.bfloat16,
                               kind="ExternalOutput")
        dbg_y = nc.dram_tensor("dbg_y", [128, 2, T], mybir.dt.bfloat16,
                               kind="ExternalOutput")
        dbg_r = nc.dram_tensor("dbg_r", [1, NSP * HL, 512], mybir.dt.bfloat16,
                               kind="ExternalOutput")
        dbg_d = nc.dram_tensor("dbg_d", [1, NSP * HL, 512], mybir.dt.float32,
                               kind="ExternalOutput")

    with tile.TileContext(nc) as tc:
        with tc.tile_pool(name="main", bufs=1) as main, \
             tc.tile_pool(name="rec", bufs=8) as recp, \
             tc.tile_pool(name="at", bufs=4) as atp, \
             tc.tile_pool(name="ob", bufs=3) as obp, \
             tc.tile_pool(name="dram", bufs=1, space="DRAM") as dram, \
             tc.tile_pool(name="sc_ps", bufs=2, space="PSUM") as sc_ps, \
             tc.tile_pool(name="av_ps", bufs=2, space="PSUM") as av_ps, \
             tc.tile_pool(name="pj_ps", bufs=2, space="PSUM") as pj_ps, \
             tc.tile_pool(name="pp_ps", bufs=2, space="PSUM") as pp_ps:

            xt_s = main.tile([128, 8, T], bf16)
            wq_s = main.tile([128, 8, DL], bf16)
            wk_s = main.tile([128, 8, DL], bf16)
            wv_s = main.tile([128, 8, DL], bf16)
            wo_s = main.tile([128, 2, D], bf16)
            qT_s = main.tile([128, 2, T], bf16)
            kT_s = main.tile([128, 2, T], bf16)
            yT_s = main.tile([128, 2, T], bf16)
            v_s = main.tile([128, 16, HL * 65], bf16)
            tri_s = main.tile([128, 128], bf16)
            bq_s = main.tile([128, 2], f32)
            bk_s = main.tile([128, 2], f32)
            bv_bc = main.tile([128, DL], bf16)
            bo_bc = main.tile([128, D], bf16)
            ones_b = main.tile([1, 64], bf16)

            partials = [dram.tile([WS[i], D], f32, name=f"partial{i}")
                        for i in range(NSP)]
            rs_outs = [dram.tile([WS[i] // GROUPS, D], f32, name=f"rsout{i}")
                       for i in range(NSP)]

            # ---- input DMAs: few large transfers, spread across queues ----
            # scalar queue: weights + small tensors (ACT is idle early)
            nc.scalar.dma_start(out=bq_s, in_=bqP[:])
            nc.scalar.dma_start(out=bk_s, in_=bkP[:])
            nc.scalar.dma_start(out=tri_s, in_=trid[:])
            wk_r = wkT[:].rearrange("(c p) n -> p c n", p=128)
            nc.scalar.dma_start(out=wk_s, in_=wk_r)
            wv_r = wvT[:].rearrange("(c p) n -> p c n", p=128)
            nc.scalar.dma_start(out=wv_s, in_=wv_r)
            nc.scalar.dma_start(out=bv_bc, in_=bv[:].to_broadcast([128, DL]))
            nc.scalar.dma_start(out=bo_bc, in_=bo[:].to_broadcast([128, D]))
            wo_r = woT[:].rearrange("(c p) n -> p c n", p=128)
            nc.scalar.dma_start(out=wo_s, in_=wo_r)
            # gpsimd queue: wq then the first x piece (critical path)
            wq_r = wqT[:].rearrange("(c p) n -> p c n", p=128)
            nc.gpsimd.dma_start(out=wq_s, in_=wq_r)
            xT_r = xT[:].rearrange("(c p) t -> p c t", p=128)
            nc.gpsimd.dma_start(out=xt_s[:, :, 0:512], in_=xT_r[:, :, 0:512])
            # sync queue: remaining x pieces
            for lo, hi in ((512, 1024), (1024, 1536), (1536, 2048)):
                nc.sync.dma_start(out=xt_s[:, :, lo:hi], in_=xT_r[:, :, lo:hi])

            nc.gpsimd.memset(ones_b, 1.0)
            nc.vector.memset(v_s, 1.0)   # ones column at index 64 per head

            # ---------------- emission helpers ----------------
            heads_ps = {}   # (qs, h) -> av psum tile awaiting evacuation
            recs = {}       # (qs, h) -> [1, W] f32 reciprocal of denominator

            def proj_q(s, w_s, b_s, dst):
                bb, ww = BS[s], WS[s]
                for mc in range(2):
                    ps = pj_ps.tile([128, 512], f32, tag="pj")
                    for kc in range(8):
                        nc.tensor.matmul(
                            ps[:, :ww],
                            lhsT=w_s[:, kc, mc * 128:(mc + 1) * 128],
                            rhs=xt_s[:, kc, bb:bb + ww],
                            start=(kc == 0), stop=(kc == 7))
                    nc.scalar.activation(
                        dst[:, mc, bb:bb + ww], ps[:, :ww], Identity,
                        bias=b_s[:, mc:mc + 1])

            def proj_v(s):
                for mt in range(BS[s] // 128, (BS[s] + WS[s]) // 128):
                    ps = pj_ps.tile([128, 512], f32, tag="pj")
                    for kc in range(8):
                        nc.tensor.matmul(
                            ps[:, :DL],
                            lhsT=xt_s[:, kc, mt * 128:(mt + 1) * 128],
                            rhs=wv_s[:, kc, :],
                            start=(kc == 0), stop=(kc == 7))
                    nc.vector.tensor_add(
                        v_s[:, mt, :].rearrange(
                            "p (h d) -> p h d", d=65)[:, :, 0:64],
                        ps[:, :DL].rearrange("p (h d) -> p h d", d=64),
                        bv_bc.rearrange("p (h d) -> p h d", d=64))

            def attn_head(qs, h):
                bb, ww = BS[qs], WS[qs]
                mc, r0 = divmod(h, 2)
                r0 *= 64
                qv = qT_s[r0:r0 + 64, mc, bb:bb + ww]
                nkt = (bb + ww) // 128
                nfull = bb // 128
                av_t = av_ps.tile([65, 512], f32, tag="av")

                def score(kt):
                    c0 = max(0, 128 * kt - bb)
                    sc_t = sc_ps.tile([128, 512], f32, tag="sc")
                    nc.tensor.matmul(
                        sc_t[:, c0:ww],
                        lhsT=kT_s[r0:r0 + 64, mc, kt * 128:(kt + 1) * 128],
                        rhs=qv[:, c0:ww], start=True, stop=True)
                    return sc_t, c0

                nxt = score(0)
                for kt in range(nkt):
                    sc_t, c0 = nxt
                    if kt + 1 < nkt:
                        nxt = score(kt + 1)  # PE runs ahead of the exp
                    at = atp.tile([128, 512], bf16, tag="at")
                    nc.scalar.activation(at[:, c0:ww], sc_t[:, c0:ww], Exp)
                    if kt >= nfull:  # diagonal tile: mask its 128-col block
                        nc.vector.tensor_mul(
                            at[:, c0:c0 + 128], at[:, c0:c0 + 128], tri_s)
                    nc.tensor.matmul(
                        av_t[:, c0:ww], lhsT=v_s[:, kt, h * 65:(h + 1) * 65],
                        rhs=at[:, c0:ww],
                        start=(kt == 0), stop=(kt == nkt - 1))
                heads_ps[(qs, h)] = av_t

            def evac(qs, h):
                bb, ww = BS[qs], WS[qs]
                mc, r0 = divmod(h, 2)
                r0 *= 64
                av_t = heads_ps.pop((qs, h))
                nc.vector.tensor_copy(
                    yT_s[r0:r0 + 64, mc, bb:bb + ww], av_t[0:64, :ww])
                den = recp.tile([1, 512], f32, tag="den")
                nc.vector.tensor_copy(den[:, :ww], av_t[64:65, :ww])
                rec = recp.tile([1, 512], f32, tag="rec")
                nc.vector.reciprocal_approx_fast(rec[:, :ww], den[:, :ww])
                rec_b = recp.tile([1, 512], bf16, tag="recb")
                nc.vector.tensor_copy(rec_b[:, :ww], rec[:, :ww])
                recs[(qs, h)] = rec_b
                if DBG:
                    den_c = recp.tile([1, 512], f32, tag="dbgden")
                    nc.vector.tensor_copy(den_c[:, :ww], av_t[64:65, :ww])
                    nc.sync.dma_start(
                        out=dbg_d[:, qs * HL + h, :ww], in_=den_c[:, :ww])
                    nc.sync.dma_start(
                        out=dbg_r[:, qs * HL + h, :ww], in_=rec_b[:, :ww])

            def post_norm(qs):
                bb, ww = BS[qs], WS[qs]
                for h in range(HL):
                    mc, r0 = divmod(h, 2)
                    r0 *= 64
                    rec = recs.pop((qs, h))
                    rb = pp_ps.tile([128, 512], f32, tag="pp")
                    nc.tensor.matmul(rb[0:64, :ww], lhsT=ones_b,
                                     rhs=rec[:, :ww],
                                     start=True, stop=True)
                    yv = yT_s[r0:r0 + 64, mc, bb:bb + ww]
                    nc.vector.tensor_mul(yv, yv, rb[0:64, :ww])

            def post_qt(qs, lq):
                qt = BS[qs] // 128 + lq
                ob = obp.tile([128, D], f32, tag="ob")
                for ns in range(2):
                    po = pp_ps.tile([128, 512], f32, tag="pp")
                    for kc in range(2):
                        nc.tensor.matmul(
                            po,
                            lhsT=yT_s[:, kc, qt * 128:(qt + 1) * 128],
                            rhs=wo_s[:, kc, ns * 512:(ns + 1) * 512],
                            start=(kc == 0), stop=(kc == 1))
                    nc.vector.tensor_add(
                        ob[:, ns * 512:(ns + 1) * 512], po,
                        bo_bc[:, ns * 512:(ns + 1) * 512])
                nc.sync.dma_start(
                    out=partials[qs][lq * 128:(lq + 1) * 128, :], in_=ob)

            def post_rs(qs):
                nc.gpsimd.collective_compute(
                    "ReduceScatter", mybir.AluOpType.add,
                    replica_groups=[[0, 1, 2, 3], [4, 5, 6, 7]],
                    ins=[partials[qs][:].opt()],
                    outs=[rs_outs[qs][:].opt()])

            # ---------------- program ----------------
            proj_q(0, wq_s, bq_s, qT_s)
            proj_q(0, wk_s, bk_s, kT_s)
            proj_v(0)

            for qs in range(NSP):
                nqt = WS[qs] // 128
                prev = qs - 1
                attn_head(qs, 0)
                if prev >= 0:
                    post_norm(prev)
                attn_head(qs, 1)
                evac(qs, 0)
                if prev >= 0:
                    for lq in range(0, min(2, WS[prev] // 128)):
                        post_qt(prev, lq)
                if qs + 1 < NSP:
                    proj_q(qs + 1, wq_s, bq_s, qT_s)
                attn_head(qs, 2)
                evac(qs, 1)
                if prev >= 0:
                    for lq in range(2, WS[prev] // 128):
                        post_qt(prev, lq)
                    post_rs(prev)
                if qs + 1 < NSP:
                    proj_q(qs + 1, wk_s, bk_s, kT_s)
                attn_head(qs, 3)
                evac(qs, 2)
                if qs + 1 < NSP:
                    proj_v(qs + 1)
                evac(qs, 3)

            last = NSP - 1
            post_norm(last)
            for lq in range(WS[last] // 128):
                post_qt(last, lq)
            post_rs(last)
            if DBG:
                nc.sync.dma_start(out=dbg_q[:], in_=qT_s)
                nc.sync.dma_start(out=dbg_k[:], in_=kT_s)
                nc.sync.dma_start(out=dbg_v[:], in_=v_s)
                nc.sync.dma_start(out=dbg_y[:], in_=yT_s)
            # final RS->out copies, all at the end of the sync queue so no
            # compute-feeding DMA ever queues behind a collective-gated one
            for qs in range(NSP):
                bb, ww = BS[qs], WS[qs]
                nc.sync.dma_start(out=out_ext[bb // 4:(bb + ww) // 4, :],
                                  in_=rs_outs[qs][:])

    nc.compile()
    return nc


def _get_program():
    if "nc" not in _CACHE:
        _CACHE["nc"] = _build_program()
    return _CACHE["nc"]


def _make_in_maps(x, mask, Wq, bq, Wk, bk, Wv, bv, Wo, bo):
    x = np.asarray(x, np.float32)
    Wq = np.asarray(Wq, np.float32)
    Wk = np.asarray(Wk, np.float32)
    Wv = np.asarray(Wv, np.float32)
    Wo = np.asarray(Wo, np.float32)
    bq = np.asarray(bq, np.float32)
    bk = np.asarray(bk, np.float32)
    bv = np.asarray(bv, np.float32)
    bo = np.asarray(bo, np.float32)

    tri = np.triu(np.ones((128, 128), np.float32)).astype(BF16)
    zeros_bo = np.zeros((1, D), np.float32)
    in_maps = []
    xTb = {b: np.ascontiguousarray(x[b].T) for b in range(B)}
    for c in range(NCORES):
        b, g = divmod(c, GROUPS)
        sl = slice(g * DL, (g + 1) * DL)
        in_maps.append({
            "xT": xTb[b].astype(BF16),
            "wqT": np.ascontiguousarray((Wq[sl] * SCALE).T).astype(BF16),
            "wkT": np.ascontiguousarray(Wk[sl].T).astype(BF16),
            "wvT": np.ascontiguousarray(Wv[sl].T).astype(BF16),
            "woT": np.ascontiguousarray(Wo[:, sl].T).astype(BF16),
            "bqP": np.ascontiguousarray((bq[sl] * SCALE).reshape(2, 128).T),
            "bkP": np.ascontiguousarray(bk[sl].reshape(2, 128).T),
            "bv": bv[sl].reshape(1, DL).astype(BF16),
            "bo": (bo.reshape(1, D) if g == 0 else zeros_bo).astype(BF16),
            "trid": tri,
        })
    return in_maps


def _capture_profile(nc, in_maps, tmpdir):
    """Run with NTFF capture and process the profile ourselves (the stock
    trace path can't handle the duplicate-executable NTFFs the axon relay
    produces). Returns (results, exec_time_ns|None)."""
    import glob
    import json
    import re
    import subprocess
    from trn_agent_boot.trn_boot import _ntff_profile_via_ctypes
    from concourse import bass2jax

    hook = _ntff_profile_via_ctypes("/opt/axon/libaxon_pjrt.so")
    if hook is None:
        raise RuntimeError("libaxon_pjrt.so lacks NTFF profile symbols")
    os.makedirs(tmpdir, exist_ok=True)
    with hook(tmpdir, [0]):
        results = bass2jax.run_bass_via_pjrt(nc, in_maps, n_cores=NCORES)

    ntffs = glob.glob(os.path.join(tmpdir, "*_body*-device*.ntff"))
    best, best_id = None, -1
    for f in ntffs:
        m = re.search(r"executable(\d+)-device000000", f)
        if m and int(m.group(1)) > best_id:
            best_id, best = int(m.group(1)), f
    if best is None:
        raise RuntimeError(f"no NTFF produced in {tmpdir}")
    neff = re.sub(r"-device\d+-execution-\d+\.ntff$", ".neff", best)
    out_json = os.path.join(tmpdir, "prof.json")
    subprocess.check_call(
        ["neuron-profile", "view", "--ignore-nc-buf-usage", "-s", best,
         "-n", neff, "--output-format=json", f"--output-file={out_json}"],
        cwd=tmpdir)
    summary = json.load(open(out_json))["summary"][0]
    return results, int(summary["total_time"] * 1e9)


def kernel(x, mask, Wq, bq, Wk, bk, Wv, bv, Wo, bo):
    from concourse import bass_utils

    in_maps = _make_in_maps(x, mask, Wq, bq, Wk, bk, Wv, bv, Wo, bo)
    nc = _get_program()

    trace = bool(int(os.environ.get("MHA_TRACE", "0")))
    tmpdir = os.environ.get("MHA_TRACE_DIR") or None
    results = None
    if trace and tmpdir:
        try:
            results, exec_ns = _capture_profile(nc, in_maps, tmpdir)
            _CACHE["last_exec_time_ns"] = exec_ns
        except Exception as e:  # profiling is best-effort
            print(f"profiling unavailable: {type(e).__name__}: {e}")
            results = None
    if results is None:
        results = bass_utils.run_bass_kernel_spmd(
            nc, in_maps, core_ids=list(range(NCORES))).results
        _CACHE.setdefault("last_exec_time_ns", None)

    out = np.empty((B, T, D), np.float32)
    for c in range(NCORES):
        b, g = divmod(c, GROUPS)
        o = np.asarray(results[c]["out"], np.float32)
        for qs in range(NSP):
            bb, wq4 = BS[qs], WS[qs] // 4
            out[b, bb + g * wq4: bb + (g + 1) * wq4] = \
                o[bb // 4: bb // 4 + wq4]
    return out


# revision 12
# speedup vs baseline: 1.4975x; 1.2556x over previous
"""Causal multi-head attention (B=2, T=2048, D=1024, H=16) on 8 TRN2 NeuronCores.

Sharding: core c = (batch b = c//4, head-group g = c%4); each core owns 4 heads
(256 dims) of one batch. Partial out-projections are summed with one fp16
ReduceScatter per q-span over each batch's 4 cores.

v2 design (vs the 352us baseline):
  - software-pipelined score->exp->AV loop (score kt+1 issues before AV kt) so
    the PE never waits the ~630ns exp; ACT and PE run at parity.
  - causal column trimming: diagonal k-tiles only compute/exp/AV columns
    >= 128*kt - span_base; the partially-masked 128x128 block is the SAME
    upper-triangular pattern for every tile -> one 32KB tri tile replaces the
    2MB mask load.
  - 5 uneven q-spans (512,512,512,384,128): the last ReduceScatter is 4x
    smaller, shrinking the un-overlappable collective tail.
  - fp16 partials + one RS per span (half the collective bytes of fp32, and
    fp16 keeps quantization at 2^-11 so accuracy is unchanged).
  - projections of span s+1 and post-processing (normalize/out-proj/RS) of
    span s-1 are interleaved between the attention heads of span s, filling
    PE bubbles in ACT-bound spans and ACT bubbles in PE-bound spans.
  - per-head softmax denominators via DVE reciprocal_approx_fast straight from
    PSUM row 64 (the v-augmentation ones-column), broadcast across partitions
    with a rank-1 f32r matmul.
  - q/k bias evacuations on ACT (Identity+bias shares the exp table; no act
    table reloads), yT/v/out evacuations on DVE.
"""

import os
import numpy as np
import ml_dtypes

BF16 = ml_dtypes.bfloat16
FP16 = np.float16

B, T, D, H = 2, 2048, 1024, 16
HD = D // H                     # 64
NCORES = 8
GROUPS = 4                      # cores per batch (tensor-parallel degree)
HL = H // GROUPS                # heads per core = 4
DL = D // GROUPS                # dims per core = 256
SCALE = HD ** -0.5

WS = [512, 512, 512, 384, 128]  # q-span widths (sum = T)
BS = [0, 512, 1024, 1536, 1920]  # q-span base offsets
NSP = len(WS)

_CACHE = {}


def _build_program():
    import concourse.bass as bass  # noqa: F401  (registers bass machinery)
    import concourse.tile as tile
    from concourse import bacc, mybir

    f32 = mybir.dt.float32
    f32r = mybir.dt.float32r
    bf16 = mybir.dt.bfloat16
    fp16 = mybir.dt.float16
    Exp = mybir.ActivationFunctionType.Exp
    Identity = mybir.ActivationFunctionType.Identity

    nc = bacc.Bacc("TRN2", target_bir_lowering=False, debug=False,
                   num_devices=NCORES)

    xT = nc.dram_tensor("xT", [D, T], bf16, kind="ExternalInput")
    wqT = nc.dram_tensor("wqT", [D, DL], bf16, kind="ExternalInput")
    wkT = nc.dram_tensor("wkT", [D, DL], bf16, kind="ExternalInput")
    wvT = nc.dram_tensor("wvT", [D, DL], bf16, kind="ExternalInput")
    woT = nc.dram_tensor("woT", [DL, D], bf16, kind="ExternalInput")
    bqP = nc.dram_tensor("bqP", [128, 2], f32, kind="ExternalInput")
    bkP = nc.dram_tensor("bkP", [128, 2], f32, kind="ExternalInput")
    bv = nc.dram_tensor("bv", [1, DL], bf16, kind="ExternalInput")
    bo = nc.dram_tensor("bo", [1, D], bf16, kind="ExternalInput")
    trid = nc.dram_tensor("trid", [128, 128], bf16, kind="ExternalInput")
    out_ext = nc.dram_tensor("out", [T // GROUPS, D], fp16,
                             kind="ExternalOutput")
    DBG = bool(int(os.environ.get("MHA_DEBUG", "0")))
    if DBG:
        dbg_q = nc.dram_tensor("dbg_q", [128, 2, T], mybir.dt.bfloat16,
                               kind="ExternalOutput")
        dbg_k = nc.dram_tensor("dbg_k", [128, 2, T], mybir.dt.bfloat16,
                               kind="ExternalOutput")
        dbg_v = nc.dram_tensor("dbg_v", [128, 16, HL * 65], mybir.dt.bfloat16,
                               kind="ExternalOutput")
        dbg_y = nc.dram_tensor("dbg_y", [128, 2, T], mybir.dt.bfloat16,
                               kind="ExternalOutput")
        dbg_r = nc.dram_tensor("dbg_r", [1, NSP * HL, 512], mybir.dt.bfloat16,
                               kind="ExternalOutput")
        dbg_d = nc.dram_tensor("dbg_d", [1, NSP * HL, 512], mybir.dt.float32,
                               kind="ExternalOutput")

    with tile.TileContext(nc) as tc:
        with tc.tile_pool(name="main", bufs=1) as main, \
             tc.tile_pool(name="rec", bufs=8) as recp, \
             tc.tile_pool(name="at", bufs=4) as atp, \
             tc.tile_pool(name="ob", bufs=3) as obp, \
             tc.tile_pool(name="dram", bufs=1, space="DRAM") as dram, \
             tc.tile_pool(name="sc_ps", bufs=2, space="PSUM") as sc_ps, \
             tc.tile_pool(name="av_ps", bufs=2, space="PSUM") as av_ps, \
             tc.tile_pool(name="pj_ps", bufs=2, space="PSUM") as pj_ps, \
             tc.tile_pool(name="pp_ps", bufs=2, space="PSUM") as pp_ps:

            xt_s = main.tile([128, 8, T], bf16)
            wq_s = main.tile([128, 8, DL], bf16)
            wk_s = main.tile([128, 8, DL], bf16)
            wv_s = main.tile([128, 8, DL], bf16)
            wo_s = main.tile([128, 2, D], bf16)
            qT_s = main.tile([128, 2, T], bf16)
            kT_s = main.tile([128, 2, T], bf16)
            yT_s = main.tile([128, 2, T], bf16)
            v_s = main.tile([128, 16, HL * 65], bf16)
            tri_s = main.tile([128, 128], bf16)
            bq_s = main.tile([128, 2], f32)
            bk_s = main.tile([128, 2], f32)
            bv_bc = main.tile([128, DL], bf16)
            bo_bc = main.tile([128, D], bf16)
            ones_b = main.tile([1, 64], bf16)

            partials = [dram.tile([WS[i], D], fp16, name=f"partial{i}")
                        for i in range(NSP)]
            rs_outs = [dram.tile([WS[i] // GROUPS, D], fp16, name=f"rsout{i}")
                       for i in range(NSP)]

            # ---- input DMAs: few large transfers, spread across queues ----
            # scalar queue: weights + small tensors (ACT is idle early)
            nc.scalar.dma_start(out=bq_s, in_=bqP[:])
            nc.scalar.dma_start(out=bk_s, in_=bkP[:])
            nc.scalar.dma_start(out=tri_s, in_=trid[:])
            wk_r = wkT[:].rearrange("(c p) n -> p c n", p=128)
            nc.scalar.dma_start(out=wk_s, in_=wk_r)
            wv_r = wvT[:].rearrange("(c p) n -> p c n", p=128)
            nc.scalar.dma_start(out=wv_s, in_=wv_r)
            nc.scalar.dma_start(out=bv_bc, in_=bv[:].to_broadcast([128, DL]))
            nc.scalar.dma_start(out=bo_bc, in_=bo[:].to_broadcast([128, D]))
            wo_r = woT[:].rearrange("(c p) n -> p c n", p=128)
            nc.scalar.dma_start(out=wo_s, in_=wo_r)
            # gpsimd queue: wq then the first x piece (critical path)
            wq_r = wqT[:].rearrange("(c p) n -> p c n", p=128)
            nc.gpsimd.dma_start(out=wq_s, in_=wq_r)
            xT_r = xT[:].rearrange("(c p) t -> p c t", p=128)
            nc.gpsimd.dma_start(out=xt_s[:, :, 0:512], in_=xT_r[:, :, 0:512])
            # sync queue: remaining x pieces
            for lo, hi in ((512, 1024), (1024, 1536), (1536, 2048)):
                nc.sync.dma_start(out=xt_s[:, :, lo:hi], in_=xT_r[:, :, lo:hi])

            nc.gpsimd.memset(ones_b, 1.0)
            nc.vector.memset(v_s, 1.0)   # ones column at index 64 per head

            # ---------------- emission helpers ----------------
            heads_ps = {}   # (qs, h) -> av psum tile awaiting evacuation
            recs = {}       # (qs, h) -> [1, W] f32 reciprocal of denominator

            def proj_q(s, w_s, b_s, dst):
                bb, ww = BS[s], WS[s]
                for mc in range(2):
                    ps = pj_ps.tile([128, 512], f32, tag="pj")
                    for kc in range(8):
                        nc.tensor.matmul(
                            ps[:, :ww],
                            lhsT=w_s[:, kc, mc * 128:(mc + 1) * 128],
                            rhs=xt_s[:, kc, bb:bb + ww],
                            start=(kc == 0), stop=(kc == 7))
                    nc.scalar.activation(
                        dst[:, mc, bb:bb + ww], ps[:, :ww], Identity,
                        bias=b_s[:, mc:mc + 1])

            def proj_v(s):
                for mt in range(BS[s] // 128, (BS[s] + WS[s]) // 128):
                    ps = pj_ps.tile([128, 512], f32, tag="pj")
                    for kc in range(8):
                        nc.tensor.matmul(
                            ps[:, :DL],
                            lhsT=xt_s[:, kc, mt * 128:(mt + 1) * 128],
                            rhs=wv_s[:, kc, :],
                            start=(kc == 0), stop=(kc == 7))
                    nc.vector.tensor_add(
                        v_s[:, mt, :].rearrange(
                            "p (h d) -> p h d", d=65)[:, :, 0:64],
                        ps[:, :DL].rearrange("p (h d) -> p h d", d=64),
                        bv_bc.rearrange("p (h d) -> p h d", d=64))

            def attn_head(qs, h):
                bb, ww = BS[qs], WS[qs]
                mc, r0 = divmod(h, 2)
                r0 *= 64
                qv = qT_s[r0:r0 + 64, mc, bb:bb + ww]
                nkt = (bb + ww) // 128
                nfull = bb // 128
                av_t = av_ps.tile([65, 512], f32, tag="av")

                def score(kt):
                    c0 = max(0, 128 * kt - bb)
                    sc_t = sc_ps.tile([128, 512], f32, tag="sc")
                    nc.tensor.matmul(
                        sc_t[:, c0:ww],
                        lhsT=kT_s[r0:r0 + 64, mc, kt * 128:(kt + 1) * 128],
                        rhs=qv[:, c0:ww], start=True, stop=True)
                    return sc_t, c0

                nxt = score(0)
                for kt in range(nkt):
                    sc_t, c0 = nxt
                    if kt + 1 < nkt:
                        nxt = score(kt + 1)  # PE runs ahead of the exp
                    at = atp.tile([128, 512], bf16, tag="at")
                    nc.scalar.activation(at[:, c0:ww], sc_t[:, c0:ww], Exp)
                    if kt >= nfull:  # diagonal tile: mask its 128-col block
                        nc.vector.tensor_mul(
                            at[:, c0:c0 + 128], at[:, c0:c0 + 128], tri_s)
                    nc.tensor.matmul(
                        av_t[:, c0:ww], lhsT=v_s[:, kt, h * 65:(h + 1) * 65],
                        rhs=at[:, c0:ww],
                        start=(kt == 0), stop=(kt == nkt - 1))
                heads_ps[(qs, h)] = av_t

            def evac(qs, h):
                bb, ww = BS[qs], WS[qs]
                mc, r0 = divmod(h, 2)
                r0 *= 64
                av_t = heads_ps.pop((qs, h))
                nc.vector.tensor_copy(
                    yT_s[r0:r0 + 64, mc, bb:bb + ww], av_t[0:64, :ww])
                den = recp.tile([1, 512], f32, tag="den")
                nc.vector.tensor_copy(den[:, :ww], av_t[64:65, :ww])
                rec = recp.tile([1, 512], f32, tag="rec")
                nc.vector.reciprocal_approx_fast(rec[:, :ww], den[:, :ww])
                rec_b = recp.tile([1, 512], bf16, tag="recb")
                nc.vector.tensor_copy(rec_b[:, :ww], rec[:, :ww])
                recs[(qs, h)] = rec_b
                if DBG:
                    den_c = recp.tile([1, 512], f32, tag="dbgden")
                    nc.vector.tensor_copy(den_c[:, :ww], av_t[64:65, :ww])
                    nc.sync.dma_start(
                        out=dbg_d[:, qs * HL + h, :ww], in_=den_c[:, :ww])
                    nc.sync.dma_start(
                        out=dbg_r[:, qs * HL + h, :ww], in_=rec_b[:, :ww])

            def post_norm(qs):
                bb, ww = BS[qs], WS[qs]
                for h in range(HL):
                    mc, r0 = divmod(h, 2)
                    r0 *= 64
                    rec = recs.pop((qs, h))
                    rb = pp_ps.tile([128, 512], f32, tag="pp")
                    nc.tensor.matmul(rb[0:64, :ww], lhsT=ones_b,
                                     rhs=rec[:, :ww],
                                     start=True, stop=True)
                    yv = yT_s[r0:r0 + 64, mc, bb:bb + ww]
                    nc.vector.tensor_mul(yv, yv, rb[0:64, :ww])

            def post_qt(qs, lq):
                qt = BS[qs] // 128 + lq
                ob = obp.tile([128, D], fp16, tag="ob")
                for ns in range(2):
                    po = pp_ps.tile([128, 512], f32, tag="pp")
                    for kc in range(2):
                        nc.tensor.matmul(
                            po,
                            lhsT=yT_s[:, kc, qt * 128:(qt + 1) * 128],
                            rhs=wo_s[:, kc, ns * 512:(ns + 1) * 512],
                            start=(kc == 0), stop=(kc == 1))
                    nc.vector.tensor_add(
                        ob[:, ns * 512:(ns + 1) * 512], po,
                        bo_bc[:, ns * 512:(ns + 1) * 512])
                nc.sync.dma_start(
                    out=partials[qs][lq * 128:(lq + 1) * 128, :], in_=ob)

            def post_rs(qs):
                nc.gpsimd.collective_compute(
                    "ReduceScatter", mybir.AluOpType.add,
                    replica_groups=[[0, 1, 2, 3], [4, 5, 6, 7]],
                    ins=[partials[qs][:].opt()],
                    outs=[rs_outs[qs][:].opt()])

            # ---------------- program ----------------
            proj_q(0, wq_s, bq_s, qT_s)
            proj_q(0, wk_s, bk_s, kT_s)
            proj_v(0)

            for qs in range(NSP):
                nqt = WS[qs] // 128
                prev = qs - 1
                attn_head(qs, 0)
                if prev >= 0:
                    post_norm(prev)
                attn_head(qs, 1)
                evac(qs, 0)
                if prev >= 0:
                    for lq in range(0, min(2, WS[prev] // 128)):
                        post_qt(prev, lq)
                if qs + 1 < NSP:
                    proj_q(qs + 1, wq_s, bq_s, qT_s)
                attn_head(qs, 2)
                evac(qs, 1)
                if prev >= 0:
                    for lq in range(2, WS[prev] // 128):
                        post_qt(prev, lq)
                    post_rs(prev)
                if qs + 1 < NSP:
                    proj_q(qs + 1, wk_s, bk_s, kT_s)
                attn_head(qs, 3)
                evac(qs, 2)
                if qs + 1 < NSP:
                    proj_v(qs + 1)
                evac(qs, 3)

            last = NSP - 1
            post_norm(last)
            for lq in range(WS[last] // 128):
                post_qt(last, lq)
            post_rs(last)
            if DBG:
                nc.sync.dma_start(out=dbg_q[:], in_=qT_s)
                nc.sync.dma_start(out=dbg_k[:], in_=kT_s)
                nc.sync.dma_start(out=dbg_v[:], in_=v_s)
                nc.sync.dma_start(out=dbg_y[:], in_=yT_s)
            # final RS->out copies, all at the end of the sync queue so no
            # compute-feeding DMA ever queues behind a collective-gated one
            for qs in range(NSP):
                bb, ww = BS[qs], WS[qs]
                nc.sync.dma_start(out=out_ext[bb // 4:(bb + ww) // 4, :],
                                  in_=rs_outs[qs][:])

    nc.compile()
    return nc


def _get_program():
    if "nc" not in _CACHE:
        _CACHE["nc"] = _build_program()
    return _CACHE["nc"]


def _make_in_maps(x, mask, Wq, bq, Wk, bk, Wv, bv, Wo, bo):
    x = np.asarray(x, np.float32)
    Wq = np.asarray(Wq, np.float32)
    Wk = np.asarray(Wk, np.float32)
    Wv = np.asarray(Wv, np.float32)
    Wo = np.asarray(Wo, np.float32)
    bq = np.asarray(bq, np.float32)
    bk = np.asarray(bk, np.float32)
    bv = np.asarray(bv, np.float32)
    bo = np.asarray(bo, np.float32)

    tri = np.triu(np.ones((128, 128), np.float32)).astype(BF16)
    zeros_bo = np.zeros((1, D), np.float32)
    in_maps = []
    xTb = {b: np.ascontiguousarray(x[b].T) for b in range(B)}
    for c in range(NCORES):
        b, g = divmod(c, GROUPS)
        sl = slice(g * DL, (g + 1) * DL)
        in_maps.append({
            "xT": xTb[b].astype(BF16),
            "wqT": np.ascontiguousarray((Wq[sl] * SCALE).T).astype(BF16),
            "wkT": np.ascontiguousarray(Wk[sl].T).astype(BF16),
            "wvT": np.ascontiguousarray(Wv[sl].T).astype(BF16),
            "woT": np.ascontiguousarray(Wo[:, sl].T).astype(BF16),
            "bqP": np.ascontiguousarray((bq[sl] * SCALE).reshape(2, 128).T),
            "bkP": np.ascontiguousarray(bk[sl].reshape(2, 128).T),
            "bv": bv[sl].reshape(1, DL).astype(BF16),
            "bo": (bo.reshape(1, D) if g == 0 else zeros_bo).astype(BF16),
            "trid": tri,
        })
    return in_maps


def _capture_profile(nc, in_maps, tmpdir):
    """Run with NTFF capture and process the profile ourselves (the stock
    trace path can't handle the duplicate-executable NTFFs the axon relay
    produces). Returns (results, exec_time_ns|None)."""
    import glob
    import json
    import re
    import subprocess
    from trn_agent_boot.trn_boot import _ntff_profile_via_ctypes
    from concourse import bass2jax

    hook = _ntff_profile_via_ctypes("/opt/axon/libaxon_pjrt.so")
    if hook is None:
        raise RuntimeError("libaxon_pjrt.so lacks NTFF profile symbols")
    os.makedirs(tmpdir, exist_ok=True)
    with hook(tmpdir, [0]):
        results = bass2jax.run_bass_via_pjrt(nc, in_maps, n_cores=NCORES)

    ntffs = glob.glob(os.path.join(tmpdir, "*_body*-device*.ntff"))
    best, best_id = None, -1
    for f in ntffs:
        m = re.search(r"executable(\d+)-device000000", f)
        if m and int(m.group(1)) > best_id:
            best_id, best = int(m.group(1)), f
    if best is None:
        raise RuntimeError(f"no NTFF produced in {tmpdir}")
    neff = re.sub(r"-device\d+-execution-\d+\.ntff$", ".neff", best)
    out_json = os.path.join(tmpdir, "prof.json")
    subprocess.check_call(
        ["neuron-profile", "view", "--ignore-nc-buf-usage", "-s", best,
         "-n", neff, "--output-format=json", f"--output-file={out_json}"],
        cwd=tmpdir)
    summary = json.load(open(out_json))["summary"][0]
    return results, int(summary["total_time"] * 1e9)


def kernel(x, mask, Wq, bq, Wk, bk, Wv, bv, Wo, bo):
    from concourse import bass_utils

    in_maps = _make_in_maps(x, mask, Wq, bq, Wk, bk, Wv, bv, Wo, bo)
    nc = _get_program()

    trace = bool(int(os.environ.get("MHA_TRACE", "0")))
    tmpdir = os.environ.get("MHA_TRACE_DIR") or None
    results = None
    if trace and tmpdir:
        try:
            results, exec_ns = _capture_profile(nc, in_maps, tmpdir)
            _CACHE["last_exec_time_ns"] = exec_ns
        except Exception as e:  # profiling is best-effort
            print(f"profiling unavailable: {type(e).__name__}: {e}")
            results = None
    if results is None:
        results = bass_utils.run_bass_kernel_spmd(
            nc, in_maps, core_ids=list(range(NCORES))).results
        _CACHE.setdefault("last_exec_time_ns", None)

    out = np.empty((B, T, D), np.float32)
    for c in range(NCORES):
        b, g = divmod(c, GROUPS)
        o = np.asarray(results[c]["out"], np.float32)
        for qs in range(NSP):
            bb, wq4 = BS[qs], WS[qs] // 4
            out[b, bb + g * wq4: bb + (g + 1) * wq4] = \
                o[bb // 4: bb // 4 + wq4]
    return out
